# revision 46
# baseline (speedup 1.0000x reference)
"""Trainium2 Bass kernel for nn_Capsule (Efficient-CapsNet style capsule layer).

Math (see reference):
    u[b,k,j,:] = x[b,j,:] @ w[k,j,:,:]            # per-(k,j) 16x16 projection
    t[b,k,:]   = sum_j u[b,k,j,:]
    l[b,k,j]   = <u[b,k,j,:], t[b,k,:]> / sqrt(D)
    c          = softmax_k(l) + bias
    s[b,k,:]   = sum_j c[b,k,j] u[b,k,j,:]
    out        = squash(s)

Sharding: the j (N=2048) contraction axis is split over 8 cores (256 j each),
so each core reads only its w slice once (4.2 MB in bf16).  Cross-core
coupling is a single 64 KB AllReduce of t; the softmax over k is core-local.
Each core emits its fully folded partial s [32,(z k)]; the host sums the 8
partials and applies the (tiny) squash.

Per-core device schedule (v4):
  All tile columns are z-major (col = z*K + k) so that k is the innermost
  (packed) index.  The c-apply then runs as a single DVE tensor_tensor in
  2x mode with c broadcast over z via a stride-0 middle axis - the old
  explicit c_rep replication pass (one F-sized ACT/Pool pass per triad) is
  gone entirely.

  phase 1: w arrives in t2-consumption-order chunks; 32 accumulating bf16
           matmuls track the stream -> t_partial[b,(z k)] -> DVE bf16 evict
           -> AllReduce in bf16 -> four replicating read-back DMAs land
           directly in t_rep_bf[128,(z k)] (no post-AR cast)
  phase 2: 22 triads (3 tiles of 4 j each; 3-bank PSUM tiles) through a
           5-stage software pipeline; at iteration gg the emission is
           B(gg-5), SM(gg-3), U(gg), L1(gg-1), L2(gg-2), C(gg-3) so every
           stage's inputs are >= 1 iteration old and no engine FIFO
           head-of-line blocks on same-iteration work:
    U:  u = 3x matmul(block-diag-x, w_tile)    # [(4j,b)=128,(z,k)] PSUM
        u_bf = ACT copy-cast PSUM->SBUF bf16
    L1: prod = u_bf * t_rep (DVE TT 2x, t broadcast over slots);
        first z pair-fold 16->8 (DVE 2x; folds combine z-halves so k stays
        the packed innermost index)
    L2: z pair-folds 8->4->2->1 on Pool -> lg
    SM: e = exp(lg/4) (ACT); Z = reduce_k(e), 1/Z (DVE)
    C:  c = e * 1/Z (Pool)
    B:  prod2 = u_bf * c_bc  (DVE TT 2x; c broadcast over the stride-0
        z axis - k innermost keeps the 2x mode; no c replication pass)
        s_acc += fold_j(prod2)  (PE ones-matmul accumulating in one PSUM
        bank across all triads)
  tail:    s_acc[32,(z k)] -> SBUF -> DRAM in two pipelined halves; host
           sums cores + squash.

PSUM budget: 2x3 banks (u triads, double buffered) + 1 (s_acc) + 1 (t) = 8.
Engine budget per triad (modeled): DVE 2.40us (prod/ph/zq/rz/prod2),
Pool 1.90us (p4/p2/lg/c), ACT 1.74us (u_bf/exp), PE 1.49us (u mm + fold).
"""

import sys

if "/opt/trn_rl_repo" not in sys.path:
    sys.path.insert(0, "/opt/trn_rl_repo")

import numpy as np

B, N, D_IN = 32, 2048, 16
K, D_OUT = 32, 16
NCORES = 8
NS = N // NCORES          # 256 local j per core
NT = NS // 4              # 64 tiles of 4 j
NT2 = NT // 2             # 32 tile pairs (8 j each) for the t matmuls
KZ = K * D_OUT            # 512
G = 3                     # tiles per triad (PSUM: 2*G + 2 == 8 banks)
NG = (NT + G - 1) // G    # 22 triads (21 full + 1 single-tile)
EPS = 1e-20

_CACHE = {}

# sim_time.py sets this to replace the AllReduce with a local DMA so the
# single-core timeline simulator can model the schedule (it adds the real
# collective latency back as a constant).  The harness path never sets it.
_SIM_AR_AS_DMA = False


def _pack_inputs(x, w, b):
    """Per-core host-side marshaling into the DMA-friendly layouts (bf16).

    All column indices are z-major: col = z*K + k.
    """
    import ml_dtypes
    bf = ml_dtypes.bfloat16
    xr = x.astype(bf).astype(np.float32)      # [B, N, D_IN]
    wr = w.astype(bf).astype(np.float32)      # [K, N, D_IN, D_OUT]

    # fold lhsT: ones at [(jr*32+b), b] - 4 stacked 32x32 identities
    fold = np.tile(np.eye(32, dtype=np.float32), (4, 1)).astype(bf)

    per_core = []
    for r in range(NCORES):
        js, je = r * NS, (r + 1) * NS
        # w_host[64h+q, t2*512 + (z*K+k)] = w[k, js+(2*t2+h)*4 + jr, i, z]
        wc = wr[:, js:je]                         # [K, NS, D_IN, D_OUT]
        wc = wc.transpose(1, 2, 3, 0)             # [NS, D_IN, D_OUT, K]  (j, i, z, k)
        wc = wc.reshape(NT, 64, KZ)               # [jt, (jr i), (z k)]
        wc = wc.reshape(NT2, 2, 64, KZ).transpose(1, 2, 0, 3)  # [h, q, t2, c]
        w_host = np.ascontiguousarray(wc.reshape(128, NT2 * KZ)).astype(bf)

        # block-diagonal x for the u matmuls:
        # bdx[64h+q, t2*128 + jr*32 + b] = x[b, j(tile,jr), i] iff q == jr*16+i
        xc = xr[:, js:je, :]                      # [B, NS, D_IN]
        xc = xc.transpose(1, 2, 0)                # [NS, D_IN, B]  (j, i, b)
        bdx = np.zeros((2, 64, NT2, 128), dtype=np.float32)   # [h, q, t2, col]
        xt4 = xc.reshape(NT2, 2, 4, D_IN, B)      # [t2, h, jr, i, b]
        for jr in range(4):
            bdx[:, jr * 16:(jr + 1) * 16, :, jr * 32:(jr + 1) * 32] = (
                xt4[:, :, jr].transpose(1, 2, 0, 3)           # [h, i, t2, b]
            )
        bdx_host = np.ascontiguousarray(bdx.reshape(128, NT2 * 128)).astype(bf)

        # dense xT for the t matmuls: xt[jj*16+i, t2*32+b] = x[b, js+t2*8+jj, i]
        xt = xc.reshape(NT2, 8, D_IN, B)          # [t2, jj, i, b]
        xt = xt.transpose(1, 2, 0, 3)             # [jj, i, t2, b]
        xt_host = np.ascontiguousarray(xt.reshape(128, NT2 * B)).astype(bf)

        per_core.append({"w": w_host, "bdx": bdx_host, "xt": xt_host,
                         "fold": fold})

    if np.any(b):
        # brep[p=(jr*32+bb), tile*K + k] = b[k, j(tile,jr)]  (replicated over bb)
        for r in range(NCORES):
            js = r * NS
            bc = b[:, js:js + NS, 0]                         # [K, NS]
            br = bc.transpose(1, 0).reshape(NT, 4, 1, K)     # [tile, jr, 1, k]
            br = np.broadcast_to(br, (NT, 4, 32, K))         # replicate over batch
            brep = br.transpose(1, 2, 0, 3).reshape(128, NT * K)
            per_core[r]["brep"] = np.ascontiguousarray(brep, dtype=np.float32)
        with_bias = True
    else:
        with_bias = False
    return per_core, with_bias


def _build(with_bias):
    from concourse import bacc, mybir
    from concourse.tile import TileContext

    f32 = mybir.dt.float32
    bf16 = mybir.dt.bfloat16

    nc = bacc.Bacc("TRN2", target_bir_lowering=False, debug=False,
                   num_devices=NCORES)
    w_in = nc.declare_dram_parameter("w", [128, NT2 * KZ], bf16, isOutput=False)
    bdx_in = nc.declare_dram_parameter("bdx", [128, NT2 * 128], bf16, isOutput=False)
    xt_in = nc.declare_dram_parameter("xt", [128, NT2 * B], bf16, isOutput=False)
    fold_in = nc.declare_dram_parameter("fold", [128, 32], bf16, isOutput=False)
    brep_in = None
    if with_bias:
        brep_in = nc.declare_dram_parameter("brep", [128, NT * K], f32, isOutput=False)
    s_out = nc.declare_dram_parameter("s_part", [32, KZ], f32, isOutput=True)

    t_ar_in = nc.dram_tensor("t_ar_in", [64, KZ], bf16)
    t_ar_out = nc.dram_tensor("t_ar_out", [64, KZ], bf16, addr_space="Shared")

    with TileContext(nc) as tc:
        with (
            tc.tile_pool(name="wp", bufs=1) as wp,
            tc.tile_pool(name="xp", bufs=1) as xp,
            tc.tile_pool(name="sp", bufs=1) as sp,
            tc.tile_pool(name="ub", bufs=8) as ub,
            tc.tile_pool(name="work", bufs=4) as work,
            tc.tile_pool(name="small", bufs=8) as small,
            tc.tile_pool(name="pu", bufs=2, space="PSUM") as pu,
            tc.tile_pool(name="pt", bufs=1, space="PSUM") as pt,
            tc.tile_pool(name="pa", bufs=1, space="PSUM") as pa,
        ):
            # ---- input DMA: w in t2-order chunks so the t-chain can consume
            # tiles as they land; xt (needed from the first t-matmul) slots in
            # after two w chunks; bdx and fold ride behind the w stream ----
            xt_sb = xp.tile([128, NT2 * B], bf16, tag="xt")
            fold_sb = xp.tile([128, 32], bf16, tag="fold")
            w_sb = wp.tile([128, NT2 * KZ], bf16, tag="w")
            bdx_sb = xp.tile([128, NT2 * 128], bf16, tag="bdx")
            WCH = 16
            wq = NT2 * KZ // WCH
            bq = NT2 * 128 // 2
            nc.sync.dma_start(out=xt_sb[:, :], in_=xt_in[:, :])
            for ci in range(WCH):
                nc.sync.dma_start(out=w_sb[:, ci * wq:(ci + 1) * wq],
                                  in_=w_in[:, ci * wq:(ci + 1) * wq])
            nc.sync.dma_start(out=bdx_sb[:, 0:bq], in_=bdx_in[:, 0:bq])

            brep_sb = None
            if with_bias:
                brep_sb = xp.tile([128, NT * K], f32, tag="brep")
                nc.sync.dma_start(out=brep_sb[:, :], in_=brep_in[:, :])

            s_acc = pa.tile([32, KZ], f32, tag="s_acc")

            ntiles = lambda g: min(G, NT - g * G)
            ufront = {}
            lfront = {}

            def emit_u(g):
                S = ntiles(g)
                F = S * KZ
                uq = pu.tile([128, G * KZ], f32, tag="uq")
                for s in range(S):
                    tile = g * G + s
                    t2, h = tile // 2, tile % 2
                    nc.tensor.matmul(uq[:, s * KZ:(s + 1) * KZ],
                                     bdx_sb[64 * h:64 * h + 64,
                                            t2 * 128:(t2 + 1) * 128],
                                     w_sb[64 * h:64 * h + 64,
                                          t2 * KZ:(t2 + 1) * KZ],
                                     start=True, stop=True)
                u_bf = ub.tile([128, G * KZ], bf16, tag="u_bf")
                nc.scalar.copy(u_bf[:, :F], uq[:, :F])
                ufront[g] = u_bf

            # ---- phase 1: partial t -> AllReduce -> t_rep_bf ----
            t_psum = pt.tile([32, KZ], f32, tag="t")
            for t2 in range(NT2):
                nc.tensor.matmul(t_psum[:, :],
                                 xt_sb[:, t2 * B:(t2 + 1) * B],
                                 w_sb[:, t2 * KZ:(t2 + 1) * KZ],
                                 start=(t2 == 0), stop=(t2 == NT2 - 1))
            # Evict the PSUM partial on DVE with a bf16 downcast (idle until
            # t arrives; GPSIMD cannot access PSUM), then duplicate it to a
            # second partition group on ACT (idle; same SBUF->SBUF cross-
            # partition copy the baseline used).  The AllReduce runs in bf16
            # on the duplicated [64,KZ] payload so only TWO replicating
            # read-back DMAs are needed to fill t_rep_bf - the four-way
            # HWDGE serialization behind the collective is halved.
            t_loc = sp.tile([64, KZ], bf16, tag="t_loc")
            nc.vector.tensor_copy(t_loc[0:32, :], t_psum[:, :])
            nc.scalar.copy(t_loc[32:64, :], t_loc[0:32, :])
            nc.sync.dma_start(out=t_ar_in[:, :], in_=t_loc[:, :])
            if _SIM_AR_AS_DMA:
                nc.sync.dma_start(out=t_ar_out[:, :], in_=t_ar_in[:, :])
            else:
                nc.gpsimd.collective_compute(
                    "AllReduce",
                    mybir.AluOpType.add,
                    replica_groups=[list(range(NCORES))],
                    ins=[t_ar_in[:, :].opt()],
                    outs=[t_ar_out[:, :].opt()],
                )
            t_rep_bf = sp.tile([128, KZ], bf16, tag="t_rep_bf")
            for q in range(2):
                # one read per HWDGE queue (SP / ACT)
                eng = nc.sync if q == 0 else nc.scalar
                eng.dma_start(out=t_rep_bf[64 * q:64 * q + 64, :],
                              in_=t_ar_out[:, :])
            # second bdx half + fold ride AFTER the t-path DMAs so the
            # critical AllReduce plumbing never queues behind bulk loads
            nc.sync.dma_start(out=bdx_sb[:, bq:], in_=bdx_in[:, bq:])
            nc.sync.dma_start(out=fold_sb[:, :], in_=fold_in[:, :])

            # ---- phase 2: 5-stage software pipeline over triads ----
            phfront = {}
            smfront = {}
            cfront = {}

            def emit_l1(g):
                # prod = u * t (t broadcast over slots) and the first z fold
                S = ntiles(g)
                F = S * KZ
                u_bf = ufront[g]
                prod = work.tile([128, G * KZ], bf16, tag="prod")
                t_bc = t_rep_bf[:, :].unsqueeze(1).to_broadcast((128, S, KZ))
                nc.vector.tensor_tensor(
                    prod[:, :F].rearrange("p (s c) -> p s c", s=S),
                    u_bf[:, :F].rearrange("p (s c) -> p s c", s=S),
                    t_bc, op=mybir.AluOpType.mult)
                ph = work.tile([128, G * KZ // 2], bf16, tag="ph")
                pv = prod[:, :F].rearrange("p (s h m) -> p s h m", h=2, m=256)
                nc.vector.tensor_tensor(
                    ph[:, :F // 2].rearrange("p (s m) -> p s m", m=256),
                    pv[:, :, 0, :], pv[:, :, 1, :], op=mybir.AluOpType.add)
                phfront[g] = ph

            def emit_l2(g):
                # remaining z folds 8->4->2->1 on Pool (k stays packed)
                S = ntiles(g)
                F, FK = S * KZ, S * K
                ph = phfront.pop(g)
                p4 = work.tile([128, G * KZ // 4], bf16, tag="p4")
                phv = ph[:, :F // 2].rearrange("p (s h m) -> p s h m", h=2, m=128)
                nc.gpsimd.tensor_tensor(
                    p4[:, :F // 4].rearrange("p (s m) -> p s m", m=128),
                    phv[:, :, 0, :], phv[:, :, 1, :], op=mybir.AluOpType.add)
                p2 = work.tile([128, G * KZ // 8], bf16, tag="p2")
                p4v = p4[:, :F // 4].rearrange("p (s h m) -> p s h m", h=2, m=64)
                nc.gpsimd.tensor_tensor(
                    p2[:, :F // 8].rearrange("p (s m) -> p s m", m=64),
                    p4v[:, :, 0, :], p4v[:, :, 1, :], op=mybir.AluOpType.add)
                lg = small.tile([128, G * K], bf16, tag="lg")
                p2v = p2[:, :F // 8].rearrange("p (s h m) -> p s h m", h=2, m=K)
                nc.gpsimd.tensor_tensor(
                    lg[:, :FK].rearrange("p (s m) -> p s m", m=K),
                    p2v[:, :, 0, :], p2v[:, :, 1, :], op=mybir.AluOpType.add)
                lfront[g] = lg

            def emit_sm(g):
                # exp + normalizer (lg is one pipeline stage old)
                S = ntiles(g)
                FK = S * K
                lg = lfront.pop(g)
                e = small.tile([128, G * K], f32, tag="e")
                nc.scalar.activation(e[:, :FK], lg[:, :FK],
                                     mybir.ActivationFunctionType.Exp,
                                     scale=0.25)
                zq = small.tile([128, G], f32, tag="zq")
                nc.vector.tensor_reduce(
                    zq[:, :S],
                    e[:, :FK].rearrange("p (s k) -> p s k", k=K),
                    axis=mybir.AxisListType.X, op=mybir.AluOpType.add)
                rz = small.tile([128, G], f32, tag="rz")
                nc.vector.reciprocal(rz[:, :S], zq[:, :S])
                smfront[g] = (e, rz)

            def emit_c(g):
                # c = e * (1/Z) on Pool, emitted last so Pool's FIFO drains
                # its ready fold work before blocking on the fresh rz
                S = ntiles(g)
                FK = S * K
                e, rz = smfront.pop(g)
                c = small.tile([128, G * K], bf16, tag="c")
                rz_bc = rz[:, :S].unsqueeze(-1).to_broadcast((128, S, K))
                nc.gpsimd.tensor_tensor(
                    c[:, :FK].rearrange("p (s k) -> p s k", k=K),
                    e[:, :FK].rearrange("p (s k) -> p s k", k=K),
                    rz_bc, op=mybir.AluOpType.mult)
                if with_bias:
                    nc.vector.tensor_tensor(
                        c[:, :FK], c[:, :FK],
                        brep_sb[:, g * G * K:g * G * K + FK],
                        op=mybir.AluOpType.add)
                cfront[g] = c

            def emit_b(g):
                S = ntiles(g)
                F = S * KZ
                u_bf = ufront.pop(g)
                c = cfront.pop(g)
                # prod2 = u * c, with c broadcast over the z axis (stride-0
                # middle dim; k innermost keeps DVE 2x throughput).  On even
                # triads the last slot runs on Pool to shave the DVE wall.
                prod2 = work.tile([128, G * KZ], bf16, tag="prod2")
                SD = S
                c_v = c[:, :S * K].rearrange("p (s k) -> p s k", k=K)
                u_v = u_bf[:, :F].rearrange("p (s z k) -> p s z k",
                                            z=D_OUT, k=K)
                p_v = prod2[:, :F].rearrange("p (s z k) -> p s z k",
                                             z=D_OUT, k=K)
                nc.vector.tensor_tensor(
                    p_v[:, :SD], u_v[:, :SD],
                    c_v[:, :SD].unsqueeze(2).to_broadcast((128, SD, D_OUT, K)),
                    op=mybir.AluOpType.mult)
                if SD < S:
                    nc.gpsimd.tensor_tensor(
                        p_v[:, SD:], u_v[:, SD:],
                        c_v[:, SD:].unsqueeze(2)
                        .to_broadcast((128, S - SD, D_OUT, K)),
                        op=mybir.AluOpType.mult)
                # fold j (partition groups) into s_acc via PE ones-matmul
                for s in range(S):
                    tile = g * G + s
                    nc.tensor.matmul(s_acc[:, :],
                                     fold_sb[:, :],
                                     prod2[:, s * KZ:(s + 1) * KZ],
                                     start=(tile == 0), stop=(tile == NT - 1))

            # Every stage's inputs are produced >= 1 iteration earlier, so no
            # engine FIFO head-of-line blocks on work emitted the same
            # iteration (the only same-iteration edges, exp->zq and zq->c,
            # land late enough in their FIFOs to be satisfied).  The prologue
            # pre-queues two l1 windows so DVE has enough buffered work to
            # ride out the pipeline-fill latency of the softmax chain.
            emit_u(0)
            emit_u(1)
            emit_l1(0)
            emit_l1(1)
            for gg in range(2, NG + 5):
                if gg >= 5:
                    emit_b(gg - 5)
                if 3 <= gg < NG + 3:
                    emit_sm(gg - 3)
                if gg < NG:
                    emit_u(gg)
                if 3 <= gg < NG + 1:
                    emit_l1(gg - 1)
                if 2 <= gg < NG + 2:
                    emit_l2(gg - 2)
                if 3 <= gg < NG + 3:
                    emit_c(gg - 3)

            # ---- tail: ship the folded partial s (two pipelined halves) ----
            s_sb = sp.tile([32, KZ], f32, tag="s_sb")
            H = KZ // 2
            nc.scalar.copy(s_sb[:, :H], s_acc[:, :H])
            nc.sync.dma_start(out=s_out[:, :H], in_=s_sb[:, :H])
            nc.scalar.copy(s_sb[:, H:], s_acc[:, H:])
            nc.sync.dma_start(out=s_out[:, H:], in_=s_sb[:, H:])

    nc.compile()
    return nc


def _get_nc(with_bias):
    key = ("nc", with_bias)
    if key not in _CACHE:
        _CACHE[key] = _build(with_bias)
    return _CACHE[key]


def _get_runner(with_bias):
    """Build (once) a cached shard_map-jitted executable for the 8-core SPMD
    kernel, mirroring bass2jax.run_bass_via_pjrt but reusable across calls."""
    key = ("runner", with_bias)
    if key in _CACHE:
        return _CACHE[key]

    import jax
    from jax.sharding import Mesh, PartitionSpec
    from jax.experimental.shard_map import shard_map
    from concourse import mybir
    from concourse.bass2jax import (_bass_exec_p, install_neuronx_cc_hook,
                                    partition_id_tensor)

    install_neuronx_cc_hook()
    nc = _get_nc(with_bias)

    partition_name = nc.partition_id_tensor.name if nc.partition_id_tensor else None
    in_names, out_names, out_avals, zero_shapes = [], [], [], []
    for alloc in nc.m.functions[0].allocations:
        if not isinstance(alloc, mybir.MemoryLocationSet):
            continue
        name = alloc.memorylocations[0].name
        if alloc.kind == "ExternalInput":
            if name != partition_name:
                in_names.append(name)
        elif alloc.kind == "ExternalOutput":
            out_names.append(name)
            shape = tuple(alloc.tensor_shape)
            dtype = mybir.dt.np(alloc.dtype)
            out_avals.append(jax.core.ShapedArray(shape, dtype))
            zero_shapes.append((shape, dtype))
    n_params = len(in_names)
    n_outs = len(out_avals)
    all_in_names = list(in_names) + list(out_names)
    if partition_name is not None:
        all_in_names.append(partition_name)

    def _body(*args):
        operands = list(args)
        if partition_name is not None:
            operands.append(partition_id_tensor())
        outs = _bass_exec_p.bind(
            *operands,
            out_avals=tuple(out_avals),
            in_names=tuple(all_in_names),
            out_names=tuple(out_names),
            lowering_input_output_aliases=(),
            sim_require_finite=True,
            sim_require_nnan=True,
            nc=nc,
        )
        return tuple(outs)

    devices = jax.devices()[:NCORES]
    mesh = Mesh(np.asarray(devices), ("core",))
    in_specs = (PartitionSpec("core"),) * (n_params + n_outs)
    out_specs = (PartitionSpec("core"),) * n_outs
    donate = tuple(range(n_params, n_params + n_outs))
    sharded = jax.jit(
        shard_map(_body, mesh=mesh, in_specs=in_specs, out_specs=out_specs,
                  check_rep=False),
        donate_argnums=donate, keep_unused=True)

    def run(per_core):
        concat_in = [
            np.concatenate([np.asarray(per_core[c][nm]) for c in range(NCORES)], axis=0)
            for nm in in_names
        ]
        concat_zeros = [np.zeros((NCORES * sh[0], *sh[1:]), dt)
                        for sh, dt in zero_shapes]
        out_arrs = sharded(*concat_in, *concat_zeros)
        return [
            {nm: np.asarray(out_arrs[i]).reshape(NCORES, *out_avals[i].shape)[c]
             for i, nm in enumerate(out_names)}
            for c in range(NCORES)
        ]

    _CACHE[key] = run
    return run


def kernel(x, w, b, _run_kwargs=None):
    x = np.asarray(x, dtype=np.float32)
    w = np.asarray(w, dtype=np.float32)
    b = np.asarray(b, dtype=np.float32)

    per_core, with_bias = _pack_inputs(x, w, b)
    results = _get_runner(with_bias)(per_core)

    s = np.zeros((32, KZ), dtype=np.float64)
    for r in range(NCORES):
        s += results[r]["s_part"].astype(np.float64)
    # columns are z-major: [32, (z k)] -> [B, K, D_OUT]
    s = s.astype(np.float32).reshape(B, D_OUT, K).transpose(0, 2, 1)

    # efficient squash (host-side finalization of the gathered partials)
    n = np.linalg.norm(s.astype(np.float64), axis=-1, keepdims=True)
    out = (1.0 - 1.0 / (np.exp(n) + EPS)) * (s / (n + EPS))
    return out.astype(np.float32)
